# revision 1
# baseline (speedup 1.0000x reference)
"""Cost-volume kernel for Trainium2.

Strategy:
  - The dominant cost of this module on-device is the two exact kNN searches
    (top-16 and top-4 over per-batch 4096x4096 distance matrices).  A Bass
    kernel computes those, sharded over 8 NeuronCores (core c handles batch
    c//2, query half c%2): PE computes the score matrix s = 2*q.x - |x|^2
    (argmax_s == argmin_dist per query row), then the Vector engine extracts
    exact top-k values+indices with max8 / match_replace / max_index.
  - The gather + tiny-MLP + training-mode-BN tail is dense matmul work that
    needs global (cross-batch) BN statistics; it runs as a single jitted XLA
    program on the first NeuronCore using the Bass-computed indices.
"""

import numpy as np
import jax
import jax.numpy as jnp

B, N, C_IN = 4, 4096, 64
KQ, KN = 16, 4
N_CORES = 8
QPC = N * B // N_CORES  # queries per core = 2048
EPS_BN = 1e-5
EPS_EUC = 1e-20

# ----------------------------------------------------------------------------
# Bass kNN kernel
# ----------------------------------------------------------------------------

_BASS_CACHE = {}


def _build_knn_nc():
    import concourse.mybir as mybir
    from concourse import bacc
    from concourse.tile import TileContext

    nc = bacc.Bacc("TRN2", target_bir_lowering=False)
    f32 = mybir.dt.float32
    q4 = nc.dram_tensor("q4", [4, QPC], f32, kind="ExternalInput")
    r1 = nc.dram_tensor("r1", [4, N], f32, kind="ExternalInput")
    r2 = nc.dram_tensor("r2", [4, N], f32, kind="ExternalInput")
    idx1 = nc.dram_tensor("idx1", [QPC, 16], mybir.dt.uint32, kind="ExternalOutput")
    idx2 = nc.dram_tensor("idx2", [QPC, 8], mybir.dt.uint32, kind="ExternalOutput")

    n_tiles = QPC // 128
    NEG = -3.0e38

    with TileContext(nc) as tc:
        with (
            tc.tile_pool(name="consts", bufs=1) as cpool,
            tc.tile_pool(name="psum", bufs=8, space="PSUM") as ppool,
            tc.tile_pool(name="scores", bufs=2) as spool,
            tc.tile_pool(name="small", bufs=8) as vpool,
        ):
            q_sb = cpool.tile([4, QPC], f32, tag="q")
            r1_sb = cpool.tile([4, N], f32, tag="r1")
            r2_sb = cpool.tile([4, N], f32, tag="r2")
            nc.sync.dma_start(q_sb[:], q4[:])
            nc.sync.dma_start(r1_sb[:], r1[:])
            nc.sync.dma_start(r2_sb[:], r2[:])

            for t in range(n_tiles):
                lhsT = q_sb[:, t * 128:(t + 1) * 128]
                for stage, (r_sb, kidx, kout) in enumerate(
                    [(r1_sb, idx1, 16), (r2_sb, idx2, 8)]
                ):
                    s_sb = spool.tile([128, N], f32, tag="s")
                    for j in range(N // 512):
                        ps = ppool.tile([128, 512], f32, tag="ps")
                        nc.tensor.matmul(
                            ps[:], lhsT, r_sb[:, j * 512:(j + 1) * 512],
                            start=True, stop=True,
                        )
                        nc.scalar.copy(s_sb[:, j * 512:(j + 1) * 512], ps[:])
                    v8a = vpool.tile([128, 8], f32, tag="v")
                    i8a = vpool.tile([128, 8], mybir.dt.uint32, tag="i")
                    nc.vector.max(out=v8a[:], in_=s_sb[:])
                    nc.vector.max_index(out=i8a[:], in_max=v8a[:], in_values=s_sb[:])
                    nc.sync.dma_start(kidx[t * 128:(t + 1) * 128, 0:8], i8a[:])
                    if kout == 16:
                        v8b = vpool.tile([128, 8], f32, tag="v")
                        i8b = vpool.tile([128, 8], mybir.dt.uint32, tag="i")
                        nc.vector.match_replace(
                            out=s_sb[:], in_to_replace=v8a[:], in_values=s_sb[:],
                            imm_value=NEG,
                        )
                        nc.vector.max(out=v8b[:], in_=s_sb[:])
                        nc.vector.max_index(
                            out=i8b[:], in_max=v8b[:], in_values=s_sb[:]
                        )
                        nc.sync.dma_start(kidx[t * 128:(t + 1) * 128, 8:16], i8b[:])
    nc.compile()
    return nc


def _make_runner(nc, n_cores=N_CORES):
    """Build a cached jitted shard_map runner for the Bass NEFF (mirrors
    concourse.bass2jax.run_bass_via_pjrt but reusable across calls)."""
    import concourse.mybir as mybir
    from concourse.bass2jax import (
        _bass_exec_p,
        install_neuronx_cc_hook,
        partition_id_tensor,
    )
    from jax.sharding import Mesh, PartitionSpec
    from jax.experimental.shard_map import shard_map

    install_neuronx_cc_hook()
    partition_name = (
        nc.partition_id_tensor.name if nc.partition_id_tensor else None
    )
    in_names, out_names, out_avals, zero_outs = [], [], [], []
    for alloc in nc.m.functions[0].allocations:
        if not isinstance(alloc, mybir.MemoryLocationSet):
            continue
        name = alloc.memorylocations[0].name
        if alloc.kind == "ExternalInput":
            if name != partition_name:
                in_names.append(name)
        elif alloc.kind == "ExternalOutput":
            shape = tuple(alloc.tensor_shape)
            dtype = mybir.dt.np(alloc.dtype)
            out_names.append(name)
            out_avals.append(jax.core.ShapedArray(shape, dtype))
            zero_outs.append(np.zeros(shape, dtype))
    n_params = len(in_names)
    n_outs = len(out_avals)
    all_in_names = list(in_names) + list(out_names)
    if partition_name is not None:
        all_in_names.append(partition_name)
    donate = tuple(range(n_params, n_params + n_outs))

    def _body(*args):
        operands = list(args)
        if partition_name is not None:
            operands.append(partition_id_tensor())
        outs = _bass_exec_p.bind(
            *operands,
            out_avals=tuple(out_avals),
            in_names=tuple(all_in_names),
            out_names=tuple(out_names),
            lowering_input_output_aliases=(),
            sim_require_finite=True,
            sim_require_nnan=True,
            nc=nc,
        )
        return tuple(outs)

    devices = jax.devices()[:n_cores]
    mesh = Mesh(np.asarray(devices), ("core",))
    in_specs = (PartitionSpec("core"),) * (n_params + n_outs)
    out_specs = (PartitionSpec("core"),) * n_outs
    sharded = jax.jit(
        shard_map(_body, mesh=mesh, in_specs=in_specs, out_specs=out_specs,
                  check_rep=False),
        donate_argnums=donate, keep_unused=True,
    )

    def run(in_maps):
        concat_in = [
            np.concatenate([np.asarray(m[name]) for m in in_maps], axis=0)
            for name in in_names
        ]
        concat_zeros = [
            np.zeros((n_cores * z.shape[0], *z.shape[1:]), z.dtype)
            for z in zero_outs
        ]
        out_arrs = sharded(*concat_in, *concat_zeros)
        out_arrs = [np.asarray(a) for a in out_arrs]
        return [
            {
                name: out_arrs[i].reshape(n_cores, *out_avals[i].shape)[c]
                for i, name in enumerate(out_names)
            }
            for c in range(n_cores)
        ]

    return run


def _knn_runner():
    if "runner" not in _BASS_CACHE:
        nc = _build_knn_nc()
        _BASS_CACHE["runner"] = _make_runner(nc)
    return _BASS_CACHE["runner"]


def _bass_knn(warped_xyz, f2_xyz):
    """warped_xyz, f2_xyz: [B, N, 3] float32 numpy.
    Returns idx_q [B, N, 16] int32, idx [B, N, 4] int32."""
    run = _knn_runner()
    in_maps = []
    for c in range(N_CORES):
        b, h = c // 2, c % 2
        q = warped_xyz[b, h * QPC:(h + 1) * QPC]          # [2048, 3]
        q4 = np.concatenate(
            [q.T, np.ones((1, QPC), np.float32)], axis=0
        ).astype(np.float32)                               # [4, 2048]
        rs = []
        for cand in (f2_xyz[b], warped_xyz[b]):            # [4096, 3]
            sq = np.sum(cand.astype(np.float32) ** 2, axis=1, dtype=np.float32)
            rs.append(np.concatenate(
                [2.0 * cand.T, -sq[None, :]], axis=0
            ).astype(np.float32))                          # [4, 4096]
        in_maps.append({"q4": q4, "r1": rs[0], "r2": rs[1]})
    res = run(in_maps)
    idx_q = np.zeros((B, N, 16), np.int32)
    idx = np.zeros((B, N, 4), np.int32)
    for c in range(N_CORES):
        b, h = c // 2, c % 2
        sl = slice(h * QPC, (h + 1) * QPC)
        idx_q[b, sl] = res[c]["idx1"].astype(np.int32)
        idx[b, sl] = res[c]["idx2"][:, :4].astype(np.int32)
    return idx_q, idx


# ----------------------------------------------------------------------------
# jax tail: gathers + MLPs + training-mode BN + softmax aggregation
# ----------------------------------------------------------------------------

def _bn_relu(x, gamma, beta):
    m = jnp.mean(x, axis=(0, 1, 2))
    v = jnp.var(x, axis=(0, 1, 2))
    return jax.nn.relu((x - m) * jax.lax.rsqrt(v + EPS_BN) * gamma + beta)


_gather = jax.vmap(lambda p, i: p[i])


def _mlp_tail(warped_xyz, f2_xyz, warped_points, f2_points,
              mlp1, enc1, mlp2, enc2, mlp3, idx_q, idx):
    qi_xyz = _gather(f2_xyz, idx_q)
    qi_pts = _gather(f2_points, idx_q)
    pi_xyz = jnp.broadcast_to(warped_xyz[:, :, None, :], qi_xyz.shape)
    pi_pts = jnp.broadcast_to(warped_points[:, :, None, :], qi_pts.shape)
    diff = qi_xyz - pi_xyz
    euc = jnp.sqrt(jnp.sum(diff ** 2, -1, keepdims=True) + EPS_EUC)
    xyz_cat = jnp.concatenate([pi_xyz, qi_xyz, diff, euc], -1)
    h = jnp.concatenate([xyz_cat, pi_pts, qi_pts], -1)
    for (W, b, g, be) in mlp1:
        h = _bn_relu(jnp.einsum('bnkc,oc->bnko', h, W) + b, g, be)
    We, be_b, g1, bt1 = enc1
    enc = _bn_relu(jnp.einsum('bnkc,oc->bnko', xyz_cat, We) + be_b, g1, bt1)
    pc = jnp.concatenate([enc, h], -1)
    for (W, b, g, be) in mlp2:
        pc = _bn_relu(jnp.einsum('bnkc,oc->bnko', pc, W) + b, g, be)
    WQ = jax.nn.softmax(pc, axis=2)
    feat1 = jnp.sum(WQ * h, axis=2)

    g_xyz = _gather(warped_xyz, idx)
    g_pts = _gather(feat1, idx)
    c_xyz = jnp.broadcast_to(warped_xyz[:, :, None, :], g_xyz.shape)
    c_pts = jnp.broadcast_to(
        warped_points[:, :, None, :],
        (g_pts.shape[0], g_pts.shape[1], KN, warped_points.shape[-1]),
    )
    diff2 = g_xyz - c_xyz
    euc2 = jnp.sqrt(jnp.sum(diff2 ** 2, -1, keepdims=True) + EPS_EUC)
    xyz_cat2 = jnp.concatenate([c_xyz, g_xyz, diff2, euc2], -1)
    We2, b2, g2, bt2 = enc2
    enc_2 = _bn_relu(jnp.einsum('bnkc,oc->bnko', xyz_cat2, We2) + b2, g2, bt2)
    x = jnp.concatenate([enc_2, c_pts, g_pts], -1)
    for (Wc, bc), (_, _, gm, bm) in zip(mlp3, mlp2):
        x = _bn_relu(jnp.einsum('bnkc,oc->bnko', x, Wc) + bc, gm, bm)
    WP = jax.nn.softmax(x, axis=2)
    return jnp.sum(WP * g_pts, axis=2)


def _mlp_tail_fn():
    if "mlp" not in _BASS_CACHE:
        _BASS_CACHE["mlp"] = jax.jit(_mlp_tail)
    return _BASS_CACHE["mlp"]


# ----------------------------------------------------------------------------
# entry point
# ----------------------------------------------------------------------------

def kernel(warped_xyz, f2_xyz, warped_points, f2_points, mlp1, enc1, mlp2,
           enc2, mlp3):
    wxyz = np.asarray(warped_xyz, np.float32)
    fxyz = np.asarray(f2_xyz, np.float32)
    idx_q, idx = _bass_knn(wxyz, fxyz)
    f = _mlp_tail_fn()
    out = f(jnp.asarray(wxyz), jnp.asarray(fxyz),
            jnp.asarray(np.asarray(warped_points, np.float32)),
            jnp.asarray(np.asarray(f2_points, np.float32)),
            mlp1, enc1, mlp2, enc2, mlp3,
            jnp.asarray(idx_q), jnp.asarray(idx))
    return np.asarray(out)


# revision 6
# speedup vs baseline: 2.8291x; 2.8291x over previous
"""Cost-volume kernel for Trainium2.

Strategy:
  - The dominant cost of this module on-device is the two exact kNN searches
    (top-16 and top-4 over per-batch 4096x4096 distance matrices).  A Bass
    kernel computes those, sharded over 8 NeuronCores (core c handles batch
    c//2, query half c%2): the PE computes e = q.x (K=3 matmul), ACT scales
    to 2e on PSUM eviction, DVE forms s = 2e - (|q|^2 + |x|^2) = -dist
    (bit-matching the reference's summation order), then extracts exact
    top-k values+indices with max8 / match_replace / max_index.
  - The gather + tiny-MLP + training-mode-BN tail needs global (cross-batch)
    BN statistics; it runs as a single jitted XLA program using the
    Bass-computed indices, which stay resident on device between the two
    dispatches.
"""

import numpy as np
import jax
import jax.numpy as jnp

B, N, C_IN = 4, 4096, 64
KQ, KN = 16, 4
N_CORES = 8
QPC = N * B // N_CORES  # queries per core = 2048
N_TILES = QPC // 128
EPS_BN = 1e-5
EPS_EUC = 1e-20

_CACHE = {}

# ----------------------------------------------------------------------------
# Bass kNN kernel
# ----------------------------------------------------------------------------


def _build_knn_nc():
    import concourse.mybir as mybir
    from concourse import bacc
    from concourse.tile import TileContext

    nc = bacc.Bacc("TRN2", target_bir_lowering=False)
    f32 = mybir.dt.float32
    q3 = nc.dram_tensor("q3", [3, QPC], f32, kind="ExternalInput")
    sqt = nc.dram_tensor("sqt", [128, N_TILES], f32, kind="ExternalInput")
    r1 = nc.dram_tensor("r1", [3, N], f32, kind="ExternalInput")
    sx1 = nc.dram_tensor("sx1", [128, N], f32, kind="ExternalInput")
    r2 = nc.dram_tensor("r2", [3, N], f32, kind="ExternalInput")
    sx2 = nc.dram_tensor("sx2", [128, N], f32, kind="ExternalInput")
    idx1 = nc.dram_tensor("idx1", [QPC, 16], mybir.dt.uint32, kind="ExternalOutput")
    idx2 = nc.dram_tensor("idx2", [QPC, 8], mybir.dt.uint32, kind="ExternalOutput")

    NEG = -3.0e38
    Copy = mybir.ActivationFunctionType.Copy

    with TileContext(nc) as tc:
        with (
            tc.tile_pool(name="consts", bufs=1) as cpool,
            tc.tile_pool(name="psum", bufs=8, space="PSUM") as ppool,
            tc.tile_pool(name="scores", bufs=2) as spool,
            tc.tile_pool(name="small", bufs=8) as vpool,
        ):
            q_sb = cpool.tile([3, QPC], f32, tag="q")
            sqt_sb = cpool.tile([128, N_TILES], f32, tag="sqt")
            r1_sb = cpool.tile([3, N], f32, tag="r1")
            sx1_sb = cpool.tile([128, N], f32, tag="sx1")
            r2_sb = cpool.tile([3, N], f32, tag="r2")
            sx2_sb = cpool.tile([128, N], f32, tag="sx2")
            for sb, dr in [(q_sb, q3), (sqt_sb, sqt), (r1_sb, r1),
                           (sx1_sb, sx1), (r2_sb, r2), (sx2_sb, sx2)]:
                nc.sync.dma_start(sb[:], dr[:])

            for t in range(N_TILES):
                lhsT = q_sb[:, t * 128:(t + 1) * 128]
                for r_sb, sx_sb, kidx, kout in (
                    (r1_sb, sx1_sb, idx1, 16),
                    (r2_sb, sx2_sb, idx2, 8),
                ):
                    s_sb = spool.tile([128, N], f32, tag="s")
                    for j in range(N // 512):
                        ps = ppool.tile([128, 512], f32, tag="ps")
                        nc.tensor.matmul(
                            ps[:], lhsT, r_sb[:, j * 512:(j + 1) * 512],
                            start=True, stop=True,
                        )
                        # s <- 2*e  (exact: scale by power of two)
                        nc.scalar.activation(
                            s_sb[:, j * 512:(j + 1) * 512], ps[:], Copy,
                            scale=2.0,
                        )
                    # t1 <- |q|^2 + |x|^2  (same operand order as reference)
                    t1 = spool.tile([128, N], f32, tag="t1")
                    nc.vector.tensor_scalar(
                        out=t1[:],
                        in0=sx_sb[:],
                        scalar1=sqt_sb[:, t:t + 1],
                        scalar2=None,
                        op0=mybir.AluOpType.add,
                    )
                    # s <- 2e - (|q|^2+|x|^2) = -dist  (negation-exact)
                    nc.vector.tensor_sub(s_sb[:], s_sb[:], t1[:])

                    v8a = vpool.tile([128, 8], f32, tag="v")
                    i8a = vpool.tile([128, 8], mybir.dt.uint32, tag="i")
                    nc.vector.max(out=v8a[:], in_=s_sb[:])
                    nc.vector.max_index(out=i8a[:], in_max=v8a[:], in_values=s_sb[:])
                    nc.sync.dma_start(kidx[t * 128:(t + 1) * 128, 0:8], i8a[:])
                    if kout == 16:
                        v8b = vpool.tile([128, 8], f32, tag="v")
                        i8b = vpool.tile([128, 8], mybir.dt.uint32, tag="i")
                        nc.vector.match_replace(
                            out=s_sb[:], in_to_replace=v8a[:], in_values=s_sb[:],
                            imm_value=NEG,
                        )
                        nc.vector.max(out=v8b[:], in_=s_sb[:])
                        nc.vector.max_index(
                            out=i8b[:], in_max=v8b[:], in_values=s_sb[:]
                        )
                        nc.sync.dma_start(kidx[t * 128:(t + 1) * 128, 8:16], i8b[:])
    nc.compile()
    return nc


def _make_runner(nc, n_cores=N_CORES):
    """Cached jitted shard_map runner for the Bass NEFF (mirrors
    concourse.bass2jax.run_bass_via_pjrt, but reusable and returning
    device-resident outputs)."""
    import concourse.mybir as mybir
    from concourse.bass2jax import (
        _bass_exec_p,
        install_neuronx_cc_hook,
        partition_id_tensor,
    )
    from jax.sharding import Mesh, PartitionSpec
    from jax.experimental.shard_map import shard_map

    install_neuronx_cc_hook()
    partition_name = (
        nc.partition_id_tensor.name if nc.partition_id_tensor else None
    )
    in_names, out_names, out_avals, zero_outs = [], [], [], []
    for alloc in nc.m.functions[0].allocations:
        if not isinstance(alloc, mybir.MemoryLocationSet):
            continue
        name = alloc.memorylocations[0].name
        if alloc.kind == "ExternalInput":
            if name != partition_name:
                in_names.append(name)
        elif alloc.kind == "ExternalOutput":
            shape = tuple(alloc.tensor_shape)
            dtype = mybir.dt.np(alloc.dtype)
            out_names.append(name)
            out_avals.append(jax.core.ShapedArray(shape, dtype))
            zero_outs.append(np.zeros(shape, dtype))
    n_params = len(in_names)
    n_outs = len(out_avals)
    all_in_names = list(in_names) + list(out_names)
    if partition_name is not None:
        all_in_names.append(partition_name)
    donate = tuple(range(n_params, n_params + n_outs))

    def _body(*args):
        operands = list(args)
        if partition_name is not None:
            operands.append(partition_id_tensor())
        outs = _bass_exec_p.bind(
            *operands,
            out_avals=tuple(out_avals),
            in_names=tuple(all_in_names),
            out_names=tuple(out_names),
            lowering_input_output_aliases=(),
            sim_require_finite=True,
            sim_require_nnan=True,
            nc=nc,
        )
        return tuple(outs)

    devices = jax.devices()[:n_cores]
    mesh = Mesh(np.asarray(devices), ("core",))
    in_specs = (PartitionSpec("core"),) * (n_params + n_outs)
    out_specs = (PartitionSpec("core"),) * n_outs
    sharded = jax.jit(
        shard_map(_body, mesh=mesh, in_specs=in_specs, out_specs=out_specs,
                  check_rep=False),
        donate_argnums=donate, keep_unused=True,
    )

    def run(global_in_map):
        """global_in_map: name -> [n_cores*dim0, ...] arrays.
        Returns dict name -> device array [n_cores*dim0, ...]."""
        concat_in = [global_in_map[name] for name in in_names]
        concat_zeros = [
            np.zeros((n_cores * z.shape[0], *z.shape[1:]), z.dtype)
            for z in zero_outs
        ]
        out_arrs = sharded(*concat_in, *concat_zeros)
        return dict(zip(out_names, out_arrs))

    return run


def _knn_runner():
    if "runner" not in _CACHE:
        _CACHE["runner"] = _make_runner(_build_knn_nc())
    return _CACHE["runner"]


def _sumsq_rows(x):
    # ((x^2 + y^2) + z^2) in float32, matching jnp.sum(x**2, -1) pairwise order
    x = x.astype(np.float32)
    s = x[..., 0] * x[..., 0] + x[..., 1] * x[..., 1]
    return s + x[..., 2] * x[..., 2]


def _bass_knn_inputs(wxyz, fxyz):
    """Build the global (concatenated over 8 cores) Bass input arrays."""
    sq = _sumsq_rows(wxyz)                      # [B, N]
    sx1 = _sumsq_rows(fxyz)                     # [B, N]
    sx2 = sq
    q3, sqt, r1a, sx1a, r2a, sx2a = [], [], [], [], [], []
    for c in range(N_CORES):
        b, h = c // 2, c % 2
        sl = slice(h * QPC, (h + 1) * QPC)
        q3.append(np.ascontiguousarray(wxyz[b, sl].T))            # [3, QPC]
        sqt.append(np.ascontiguousarray(sq[b, sl].reshape(N_TILES, 128).T))
        r1a.append(np.ascontiguousarray(fxyz[b].T))               # [3, N]
        sx1a.append(np.broadcast_to(sx1[b][None, :], (128, N)))
        r2a.append(np.ascontiguousarray(wxyz[b].T))
        sx2a.append(np.broadcast_to(sx2[b][None, :], (128, N)))
    cat = lambda xs: np.concatenate(xs, axis=0).astype(np.float32)
    return {
        "q3": cat(q3), "sqt": cat(sqt), "r1": cat(r1a),
        "sx1": cat(sx1a), "r2": cat(r2a), "sx2": cat(sx2a),
    }


def _bass_knn(wxyz, fxyz):
    """Returns device arrays idx1 [8*QPC, 16] uint32, idx2 [8*QPC, 8]."""
    run = _knn_runner()
    outs = run(_bass_knn_inputs(wxyz, fxyz))
    return outs["idx1"], outs["idx2"]


# ----------------------------------------------------------------------------
# jax tail: gathers + MLPs + training-mode BN + softmax aggregation
# ----------------------------------------------------------------------------

def _bn_relu(x, gamma, beta):
    m = jnp.mean(x, axis=(0, 1, 2))
    v = jnp.var(x, axis=(0, 1, 2))
    return jax.nn.relu((x - m) * jax.lax.rsqrt(v + EPS_BN) * gamma + beta)


_gather = jax.vmap(lambda p, i: p[i])


def _mlp_tail(warped_xyz, f2_xyz, warped_points, f2_points,
              mlp1, enc1, mlp2, enc2, mlp3, idx1_flat, idx2_flat):
    # Bass outputs arrive core-major = [b-major, half-major] = flat (B, N)
    idx_q = idx1_flat.astype(jnp.int32).reshape(B, N, 16)
    idx = idx2_flat[:, :KN].astype(jnp.int32).reshape(B, N, KN)

    qi_xyz = _gather(f2_xyz, idx_q)
    qi_pts = _gather(f2_points, idx_q)
    pi_xyz = jnp.broadcast_to(warped_xyz[:, :, None, :], qi_xyz.shape)
    pi_pts = jnp.broadcast_to(warped_points[:, :, None, :], qi_pts.shape)
    diff = qi_xyz - pi_xyz
    euc = jnp.sqrt(jnp.sum(diff ** 2, -1, keepdims=True) + EPS_EUC)
    xyz_cat = jnp.concatenate([pi_xyz, qi_xyz, diff, euc], -1)
    h = jnp.concatenate([xyz_cat, pi_pts, qi_pts], -1)
    for (W, b, g, be) in mlp1:
        h = _bn_relu(jnp.einsum('bnkc,oc->bnko', h, W) + b, g, be)
    We, be_b, g1, bt1 = enc1
    enc = _bn_relu(jnp.einsum('bnkc,oc->bnko', xyz_cat, We) + be_b, g1, bt1)
    pc = jnp.concatenate([enc, h], -1)
    for (W, b, g, be) in mlp2:
        pc = _bn_relu(jnp.einsum('bnkc,oc->bnko', pc, W) + b, g, be)
    WQ = jax.nn.softmax(pc, axis=2)
    feat1 = jnp.sum(WQ * h, axis=2)

    g_xyz = _gather(warped_xyz, idx)
    g_pts = _gather(feat1, idx)
    c_xyz = jnp.broadcast_to(warped_xyz[:, :, None, :], g_xyz.shape)
    c_pts = jnp.broadcast_to(
        warped_points[:, :, None, :],
        (g_pts.shape[0], g_pts.shape[1], KN, warped_points.shape[-1]),
    )
    diff2 = g_xyz - c_xyz
    euc2 = jnp.sqrt(jnp.sum(diff2 ** 2, -1, keepdims=True) + EPS_EUC)
    xyz_cat2 = jnp.concatenate([c_xyz, g_xyz, diff2, euc2], -1)
    We2, b2, g2, bt2 = enc2
    enc_2 = _bn_relu(jnp.einsum('bnkc,oc->bnko', xyz_cat2, We2) + b2, g2, bt2)
    x = jnp.concatenate([enc_2, c_pts, g_pts], -1)
    for (Wc, bc), (_, _, gm, bm) in zip(mlp3, mlp2):
        x = _bn_relu(jnp.einsum('bnkc,oc->bnko', x, Wc) + bc, gm, bm)
    WP = jax.nn.softmax(x, axis=2)
    return jnp.sum(WP * g_pts, axis=2)


def _mlp_tail_fn():
    if "mlp" not in _CACHE:
        _CACHE["mlp"] = jax.jit(_mlp_tail)
    return _CACHE["mlp"]


# ----------------------------------------------------------------------------
# entry point
# ----------------------------------------------------------------------------

def kernel(warped_xyz, f2_xyz, warped_points, f2_points, mlp1, enc1, mlp2,
           enc2, mlp3):
    wxyz = np.asarray(warped_xyz, np.float32)
    fxyz = np.asarray(f2_xyz, np.float32)
    idx1, idx2 = _bass_knn(wxyz, fxyz)
    out = _mlp_tail_fn()(
        jnp.asarray(wxyz), jnp.asarray(fxyz),
        jnp.asarray(np.asarray(warped_points, np.float32)),
        jnp.asarray(np.asarray(f2_points, np.float32)),
        mlp1, enc1, mlp2, enc2, mlp3, idx1, idx2,
    )
    return np.asarray(out)
